# revision 16
# baseline (speedup 1.0000x reference)
"""Banded (sliding-window) multi-head attention for TRN2, 8 NeuronCores.

Problem: nn_BaseAttention (B=2, T=4096, C=512, H=8, hd=64, WIN=128).
  qkv = x @ W_qkv ; banded softmax(q k^T / sqrt(hd), |i-j|<=WIN) @ v ; @ W_out + b_out

Sharding: 8 cores = 2 batches x 4 T-chunks of 1024 queries. Each core gets its
x rows plus a 128-row halo on each side (zero-padded at sequence edges) and
full replicated weights; it produces its own [1024, 512] output slice, so the
host-side gather is pure concatenation (no cross-core reduction).

Device pipeline per core (all layouts chosen to avoid transposing activations):
  xT   = x^T pre-transposed on host                 [C, 1280]
  q^T/k^T = W_qkv-slice^T-free matmuls (lhsT = W)   [hd, rows]  (head-major)
  v    = natural matmuls (lhsT = xT)                [rows, hd]
  S^T  = k^T-stationary matmuls -> PSUM strips      [keys, qcols]
  exp on ACT (strip-wide, no bias); out-of-band entries zeroed by 128-col
  affine selects on the band crossings; out-of-sequence keys contribute
  nothing because their V rows are zero (zero-padded halo) and their ones
  column is zeroed via a per-row validity mask.
  O^T  = sum_kt V-stationary matmuls over es        [2*hd, qcols]
  sums = ones-stationary matmuls over es (64-wide replicated rows)
  O^T * recip(sums) -> O_all^T ; Y = O_all^T-stationary @ W_out  (fp16 out)
b_out is added on the host after the gather.
"""

import os
import numpy as np

import concourse.bass as bass
from concourse import bacc
import concourse.mybir as mybir
import concourse.tile as tile
from concourse.bass_utils import run_bass_kernel_spmd
from concourse.masks import make_identity

# ----- problem constants (hardcoded per contest contract) -----
B, T, C = 2, 4096, 512
H, HD, WIN = 8, 64, 128
NCORES = 8
CHUNK = 1024                # queries per core
ROWS = CHUNK + 2 * WIN      # 1280 rows incl. halo
QCW = 512                   # query-chunk width (qcols per S^T tile group)
NQC = CHUNK // QCW          # 2
NKT = (QCW + 2 * WIN) // 128  # 6 key tiles per query-chunk
SCALE = HD ** -0.5

F32 = mybir.dt.float32
F16 = mybir.dt.float16
EXP = mybir.ActivationFunctionType.Exp

# per-key-tile geometry: d = key-tile offset rel. to query-chunk start.
# tc0..tc1 = qcols that contain any in-band entry for this key tile.
# Key tiles are packed into four 384-col PSUM strips per (qc, pr, j):
#   strip 0 = kt0|kt1, strip 1 = kt2, strip 2 = kt3, strip 3 = kt4|kt5
# so each strip gets one 384-wide exp.
_KT_GEOM = []
for _kt in range(NKT):
    _d = 128 * _kt - 128
    _tc0 = max(0, _d - 128)
    _tc1 = min(QCW, _d + 256)
    _ft0 = _tc0 - _d + 128  # column offset into the band strip F
    _KT_GEOM.append((_tc0, _tc1, _ft0))
_KT_STRIP = {0: (0, 0), 1: (0, 128), 2: (1, 0), 3: (2, 0), 4: (3, 0), 5: (3, 256)}
_ST_OFF = {0: 0, 1: 384, 2: 896, 3: 1280}  # strip col offsets in the es supertile
_ST_W = {0: 384, 1: 512, 2: 384, 3: 384}   # strip widths (s1 padded for the opener)
ES_W = 1664
# AV accumulation order: kt2 opens the PSUM bank with a full-width start=True
# matmul (its es tile is zero-padded in cols [384,512)); the rest accumulate
# at true width onto initialized addresses.
_AV_ORDER = [2, 0, 1, 5, 3, 4]


def build_attention_body(tc, y, xh, wqkv, wout, valid):
    """Emit the per-core kernel. All APs are DRAM tensors.

    y     [1024, 512] out f16   xh  [512, 1280] in (halo'd x rows, pre-T)
    wqkv  [512, 1536] in (q-block pre-scaled by hd^-0.5 on host)
    wout  [512, 512]  in
    valid [128, 10]   in f32 (col rt: 1.0 if local row rt*128+p is in-sequence)
    """
    nc = tc.nc
    from contextlib import ExitStack

    with ExitStack() as ctx:
        sb = ctx.enter_context(tc.tile_pool(name="sb", bufs=1))
        pp = ctx.enter_context(tc.tile_pool(name="pp", bufs=1, space="PSUM"))

        # ---- constants / persistent tiles ----
        ones_f = sb.tile([128, 128], F32, tag="ones_f", name="ones_f")
        nc.gpsimd.memset(ones_f[:], 1.0)

        # input DMAs spread over the three DMA-capable engine queues; x
        # tiles first so the projection matmuls can start as early as
        # possible.
        qs = [nc.sync, nc.scalar, nc.gpsimd]
        xT = [sb.tile([128, ROWS], F16, tag=f"xT{i}", name=f"xT{i}") for i in range(4)]
        for ct in range(4):
            qs[ct % 3].dma_start(xT[ct][:], xh[128 * ct:128 * (ct + 1), :])
        wq_sb = []
        for i in range(4):
            w_i = sb.tile([128, 3 * C], F16, tag=f"wq{i}", name=f"wq{i}")
            wq_sb.append(w_i)
        for blk in range(3):  # q block first so projections start early
            for i in range(4):
                qs[(blk + i + 1) % 3].dma_start(
                    wq_sb[i][:, C * blk:C * (blk + 1)],
                    wqkv[128 * i:128 * (i + 1), C * blk:C * (blk + 1)])
        vd = sb.tile([128, 10], F32, tag="vd", name="vd")
        nc.gpsimd.dma_start(vd[:], valid[:])
        wo_sb = []
        for i in range(4):
            w_i = sb.tile([128, C], F16, tag=f"wo{i}", name=f"wo{i}")
            qs[(i + 2) % 3].dma_start(w_i[:], wout[128 * i:128 * (i + 1), :])
            wo_sb.append(w_i)

        qT = [sb.tile([128, CHUNK], F16, tag=f"qT{i}", name=f"qT{i}") for i in range(4)]
        kT = [sb.tile([128, ROWS], F16, tag=f"kT{i}", name=f"kT{i}") for i in range(4)]
        # fused V|ones stationary tiles: vp0 blocks = [V_h | m] for even h,
        # vp1 blocks = [m | V_h] for odd h, with m the per-row validity mask
        # (folds the softmax denominator into the AV matmul while excluding
        # out-of-sequence halo keys).
        vp0 = [sb.tile([128, C], F16, tag=f"vp0_{i}", name=f"vp0_{i}") for i in range(10)]
        vp1 = [sb.tile([128, C], F16, tag=f"vp1_{i}", name=f"vp1_{i}") for i in range(10)]
        for i in range(10):
            o0 = vp0[i][:].rearrange("p (b t c) -> p b t c", t=2, c=HD)
            o1 = vp1[i][:].rearrange("p (b t c) -> p b t c", t=2, c=HD)
            nc.gpsimd.memset(o0[:, :, 1, :], 1.0)
            nc.gpsimd.memset(o1[:, :, 0, :], 1.0)
            nc.vector.tensor_scalar_mul(o0[:, :, 1, :], o0[:, :, 1, :],
                                        vd[:, i:i + 1])
            nc.vector.tensor_scalar_mul(o1[:, :, 0, :], o1[:, :, 0, :],
                                        vd[:, i:i + 1])

        # static exp-score supertiles es[(j, buf)] [128, 1664] holding the
        # four strips; one wide fp16 mask-multiply zeroes out-of-band entries
        # (and the s1 opener padding) per (qc, pr, j).
        ESB = 3
        es = {}
        for j in range(2):
            for bf in range(ESB):
                t_e = sb.tile([128, ES_W], F16, tag=f"es{j}_{bf}",
                              name=f"es{j}_{bf}")
                es[(j, bf)] = t_e
                # opener padding must start finite: NaN * 0 = NaN in the
                # mask multiply
                nc.gpsimd.memset(t_e[:, _ST_OFF[1] + 384:_ST_OFF[1] + 512], 0.0)

        # PE warm-up: dummy matmuls during the DMA prologue so the HAM clock
        # gate ramps before the real matmuls arrive.
        warm = pp.tile([128, 128], F32, tag="gp", bufs=4, name="warm")
        for _ in range(16):
            nc.tensor.matmul(warm[:], ones_f[:], ones_f[:], start=True, stop=True)

        # ---- emission helpers ----
        def emit_ft(ft):
            # q^T / k^T projection for one feature tile: out[feat, rows];
            # lhsT = W_qkv block, rhs = xT
            if ft < 4:  # q feats, own rows only (local rows [128, 1152))
                chunks = [(128, 512), (640, 512)]
                dest, doff = qT[ft], -128
            else:       # k feats, all rows
                chunks = [(0, 512), (512, 512), (1024, 256)]
                dest, doff = kT[ft - 4], 0
            for r0, rw in chunks:
                mm = pp.tile([128, QCW], F32, tag="gp", bufs=4, name="mmqk")
                for ct in range(4):
                    nc.tensor.matmul(
                        mm[:, 0:rw],
                        wq_sb[ct][:, 128 * ft:128 * (ft + 1)],
                        xT[ct][:, r0:r0 + rw],
                        start=(ct == 0), stop=(ct == 3))
                nc.vector.tensor_copy(dest[:, r0 + doff:r0 + doff + rw], mm[:, 0:rw])

        def emit_v(rt):
            # v natural projection for one row tile: out[rows, vfeat];
            # lhsT = xT tile, rhs = W_qkv v-block
            mm = pp.tile([128, QCW], F32, tag="gp", bufs=4, name="mmv")
            for ct in range(4):
                nc.tensor.matmul(
                    mm[:],
                    xT[ct][:, 128 * rt:128 * (rt + 1)],
                    wq_sb[ct][:, 1024:1536],
                    start=(ct == 0), stop=(ct == 3))
            m4 = mm[:].rearrange("p (b c) -> p b c", c=HD)
            d0 = vp0[rt][:].rearrange("p (b t c) -> p b t c", t=2, c=HD)
            d1 = vp1[rt][:].rearrange("p (b t c) -> p b t c", t=2, c=HD)
            nc.vector.tensor_copy(d0[:, :, 0, :], m4[:, 0:4, :])
            nc.vector.tensor_copy(d1[:, :, 1, :], m4[:, 4:8, :])

        # Each head-in-pair (j) gets its own fused [O^T ; sums] bank, placed
        # at partition base 64*j so the elementwise normalize stays
        # lane-aligned.
        oall = [[None] * 4 for _ in range(NQC)]
        otps = {}

        def do_s(qc, pr, j):
            # S^T strips for one head: matmul per kt into its strip columns,
            # one strip-wide exp into the supertile, then out-of-band zeroing
            # on the band-crossing 128-col windows (gpsimd is otherwise idle
            # and this pipelines per strip ahead of the AV matmuls).
            h = 2 * pr + j
            p0 = 64 * j
            e_t = es[(j, (qc * 4 + pr) % ESB)]
            for st, kts in ((0, (0, 1)), (1, (2,)), (2, (3,)), (3, (4, 5))):
                sp = pp.tile([128, 384], F32, tag="gp", bufs=4, name="sp")
                for kt in kts:
                    tc0, tc1, _ = _KT_GEOM[kt]
                    _, so = _KT_STRIP[kt]
                    kcol = 512 * qc + 128 * kt
                    nc.tensor.matmul(
                        sp[:, so:so + tc1 - tc0],
                        kT[h // 2][p0:p0 + 64, kcol:kcol + 128],
                        qT[h // 2][p0:p0 + 64,
                                   512 * qc + tc0:512 * qc + tc1],
                        start=True, stop=True)
                o = _ST_OFF[st]
                nc.scalar.activation(e_t[:, o:o + 384], sp[:], EXP)
                for kt in kts:
                    tc0, tc1, ft0 = _KT_GEOM[kt]
                    _, so = _KT_STRIP[kt]
                    off = _ST_OFF[st] + so
                    wt = tc1 - tc0
                    if ft0 < 128:   # lower bound: keep where c' >= p
                        nc.gpsimd.affine_select(
                            out=e_t[:, off:off + 128],
                            in_=e_t[:, off:off + 128],
                            compare_op=mybir.AluOpType.is_ge, fill=0.0,
                            base=0, pattern=[[1, 128]],
                            channel_multiplier=-1)
                    if ft0 + wt > 256:  # upper: keep where c'' <= p
                        ob = off + 256 - ft0
                        nc.gpsimd.affine_select(
                            out=e_t[:, ob:ob + 128],
                            in_=e_t[:, ob:ob + 128],
                            compare_op=mybir.AluOpType.is_ge, fill=0.0,
                            base=0, pattern=[[-1, 128]],
                            channel_multiplier=1)

        def do_av(qc, pr, j):
            # fused [O^T ; sums] += [V_h | m] (stationary) x es.
            # kt2 opens the bank with a full-width start=True matmul (its
            # es cols [384,512) are zero); the rest accumulate at true
            # width onto initialized addresses.
            otp = pp.tile([128, QCW], F32, tag=f"av{j}", bufs=2,
                          name=f"otp{j}")
            otps[(qc, pr, j)] = otp
            h = 2 * pr + j
            e_t = es[(j, (qc * 4 + pr) % ESB)]
            vp = vp0 if j == 0 else vp1
            for n, kt in enumerate(_AV_ORDER):
                tc0, tc1, _ = _KT_GEOM[kt]
                st, so = _KT_STRIP[kt]
                off = _ST_OFF[st] + so
                mw = 512 if kt == 2 else tc1 - tc0
                nc.tensor.matmul(
                    otp[:, tc0:tc0 + mw],
                    vp[4 * qc + kt][:, 128 * (h // 2):128 * (h // 2) + 128],
                    e_t[:, off:off + mw],
                    start=(kt == 2), stop=(n == len(_AV_ORDER) - 1),
                    skip_group_check=True)

        def do_norm(qc, pr):
            otp = [otps[(qc, pr, j)] for j in range(2)]
            oa = sb.tile([128, QCW], F16, tag=f"oa{pr}", bufs=2, name=f"oa{pr}")
            ss = sb.tile([128, QCW], F32, tag="ss", bufs=2, name="ss")
            rs = sb.tile([128, QCW], F32, tag="rs", bufs=2, name="rs")
            # sums rows sit opposite the O^T rows in each fused bank;
            # partition-base-shifted copies pack them lane-aligned (the
            # custom-DVE reciprocal can't do the PSUM re-based read itself).
            nc.vector.tensor_copy(ss[0:64, :], otp[0][64:128, :])
            nc.vector.tensor_copy(ss[64:128, :], otp[1][0:64, :])
            nc.vector.reciprocal_approx_fast(rs[:], ss[:])
            nc.vector.tensor_mul(oa[0:64, :], otp[0][0:64, :], rs[0:64, :])
            nc.vector.tensor_mul(oa[64:128, :], otp[1][64:128, :], rs[64:128, :])
            oall[qc][pr] = oa

        def do_outproj_rb(qc, rb):
            yp = pp.tile([128, C], F32, tag="gp", bufs=4, name="yp")
            for pr in range(4):
                nc.tensor.matmul(
                    yp[:],
                    oall[qc][pr][:, 128 * rb:128 * (rb + 1)],
                    wo_sb[pr][:],
                    start=(pr == 0), stop=(pr == 3))
            ys = sb.tile([128, C], F16, tag="ys", bufs=3, name="ys")
            nc.vector.tensor_copy(ys[:], yp[:])
            r0 = 512 * qc + 128 * rb
            nc.sync.dma_start(y[r0:r0 + 128, :], ys[:])

        # ---- schedule: projections interleaved with attention so the
        # PE-heavy projection work fills the ACT-bound attention stages ----
        emit_ft(0), emit_ft(4)
        do_s(0, 0, 0), do_s(0, 0, 1)
        for rt in range(6):          # v rows for qc0, while exps drain
            emit_v(rt)
        do_av(0, 0, 0), do_av(0, 0, 1), do_norm(0, 0)
        for pr in range(1, 4):
            emit_ft(pr), emit_ft(4 + pr)
            do_s(0, pr, 0), do_s(0, pr, 1)
            do_av(0, pr, 0), do_av(0, pr, 1), do_norm(0, pr)
        do_s(1, 0, 0), do_s(1, 0, 1)
        for rt in range(6, 10):      # v rows for qc1, while exps drain
            emit_v(rt)
        do_av(1, 0, 0), do_av(1, 0, 1), do_norm(1, 0)
        do_outproj_rb(0, 0)
        for pr in range(1, 4):
            do_s(1, pr, 0), do_s(1, pr, 1)
            do_av(1, pr, 0), do_av(1, pr, 1), do_norm(1, pr)
            do_outproj_rb(0, pr)
        for rb in range(4):
            do_outproj_rb(1, rb)


def build_nc():
    nc = bacc.Bacc("TRN2", target_bir_lowering=False, debug=False,
                   num_devices=NCORES)
    xh = nc.dram_tensor("xh", [C, ROWS], F16, kind="ExternalInput")
    wqkv = nc.dram_tensor("wqkv", [C, 3 * C], F16, kind="ExternalInput")
    wout = nc.dram_tensor("wout", [C, C], F16, kind="ExternalInput")
    valid = nc.dram_tensor("valid", [128, 10], F32, kind="ExternalInput")
    y = nc.dram_tensor("y", [CHUNK, C], F16, kind="ExternalOutput")
    with tile.TileContext(nc) as tc:
        build_attention_body(tc, y[:], xh[:], wqkv[:], wout[:], valid[:])
    nc.compile()
    return nc


def make_in_maps(x, W_qkv, W_out, b_out):
    """Shard the full inputs into 8 per-core input maps."""
    x = np.asarray(x, dtype=np.float32)
    wqkv = np.asarray(W_qkv, dtype=np.float32).copy()
    wqkv[:, :C] *= SCALE  # fold hd^-0.5 into the q projection
    # permute the v-block columns so the projection writes v in the fused
    # [V_even | V_odd] layout the AV matmuls consume
    wv = wqkv[:, 2 * C:3 * C].reshape(C, H, HD)
    wqkv[:, 2 * C:3 * C] = wv[:, [0, 2, 4, 6, 1, 3, 5, 7]].reshape(C, C)
    wqkv = wqkv.astype(np.float16)
    wout = np.asarray(W_out, dtype=np.float32).astype(np.float16)
    in_maps = []
    for core in range(NCORES):
        b, ch = divmod(core, 4)
        qs = CHUNK * ch
        xhalo = np.zeros((ROWS, C), dtype=np.float16)
        g0, g1 = qs - WIN, qs + CHUNK + WIN
        s0, s1 = max(g0, 0), min(g1, T)
        xhalo[s0 - g0:s1 - g0, :] = x[b, s0:s1, :].astype(np.float16)
        xhalo = np.ascontiguousarray(xhalo.T)
        # per-row validity: local row r is in-sequence iff 0 <= qs-WIN+r < T
        rows = qs - WIN + np.arange(ROWS)
        vmask = ((rows >= 0) & (rows < T)).astype(np.float32)
        valid = np.ascontiguousarray(vmask.reshape(10, 128).T)
        in_maps.append(dict(xh=xhalo, wqkv=wqkv, wout=wout, valid=valid))
    return in_maps


_CACHED_NC = None


def run_sharded(x, W_qkv, W_out, b_out, **run_kwargs):
    """Build (cached), run on 8 cores, gather. Returns (y_full, BassKernelResults)."""
    global _CACHED_NC
    if _CACHED_NC is None:
        _CACHED_NC = build_nc()
    in_maps = make_in_maps(x, W_qkv, W_out, b_out)
    res = run_bass_kernel_spmd(_CACHED_NC, in_maps, core_ids=list(range(NCORES)),
                               **run_kwargs)
    bo = np.asarray(b_out, dtype=np.float32).reshape(1, C)
    y_full = np.empty((B, T, C), dtype=np.float32)
    for core in range(NCORES):
        b, ch = divmod(core, 4)
        y_full[b, CHUNK * ch:CHUNK * (ch + 1), :] = \
            res.results[core]["y"].astype(np.float32) + bo
    return y_full, res


def kernel(x, W_qkv, W_out, b_out):
    y, _ = run_sharded(x, W_qkv, W_out, b_out)
    return y
